# revision 12
# baseline (speedup 1.0000x reference)
"""Cross-attention kernel for 8 Trainium2 NeuronCores.

Tensor-parallel over heads: 16 heads / 8 cores = 2 heads (128 channels)
per core. Each core projects q/k/v onto its 128-channel slice, runs
attention for its 2 heads, and produces a partial output projection
(x_heads @ Wo_rows). Host sums the 8 partials and adds the bias.

Device-side layout is feature-major (activations stored transposed,
[features(partitions), tokens(free)]) so every matmul contracts over the
partition dim with weights used directly as the stationary operand.
Attention uses the S^T orientation (S^T = kp^T-major) so the P@V matmul
needs no transposes; softmax denominators come for free from a ones
column appended to V (row 64 of the X accumulator), and exp() never
needs a max-subtraction because logits are bounded (~|2|) for these
inputs with scale folded into Wq.
"""

import sys

sys.path.insert(0, "/opt/trn_rl_repo")

import numpy as np

HEADS = 16
NQ = 2048
NK = 2048
DQ = 1024
DC = 768
C = 64  # head dim
NCORES = 8
HPC = HEADS // NCORES  # heads per core = 2
CW = HPC * C  # channel width per core = 128

NQT = 512  # query-token tile (matmul moving free dim)
NKT = 128  # key-token tile (S^T partition dim)
KQ = DQ // 128  # 8 contraction tiles for q projection
KC = DC // 128  # 6 contraction tiles for k/v projection
NJ = NQ // NQT  # 4 query chunks
NKC = NK // NQT  # 4 key-token chunks (projection streaming)
NT = NK // NKT  # 16 key tiles in attention

_CACHE: dict = {}


def _build():
    from contextlib import ExitStack

    from concourse import bacc, mybir, tile
    from concourse import bass as bass_mod
    from concourse.masks import make_identity

    f32 = mybir.dt.float32
    f32r = mybir.dt.float32r
    AF = mybir.ActivationFunctionType

    nc = bacc.Bacc(
        "TRN2", target_bir_lowering=False, debug=False, num_devices=NCORES
    )

    qT = nc.dram_tensor("qT", [DQ, NQ], f32r, kind="ExternalInput").ap()
    kT = nc.dram_tensor("kT", [DC, NK], f32r, kind="ExternalInput").ap()
    vT = nc.dram_tensor("vT", [DC, NK], f32r, kind="ExternalInput").ap()
    wq = nc.dram_tensor("wq", [DQ, CW], f32r, kind="ExternalInput").ap()
    wk = nc.dram_tensor("wk", [DC, CW], f32r, kind="ExternalInput").ap()
    wv = nc.dram_tensor("wv", [DC, CW], f32r, kind="ExternalInput").ap()
    wo = nc.dram_tensor("wo", [CW, DQ], f32r, kind="ExternalInput").ap()
    outT = nc.dram_tensor("outT", [DQ, NQ], f32, kind="ExternalOutput").ap()

    with tile.TileContext(nc) as tc, ExitStack() as ctx, \
         nc.allow_low_precision(reason="fp32r tiles feed the PE; 11-bit mantissa is within tolerance"):
        # ---- persistent SBUF ----
        perm = ctx.enter_context(tc.tile_pool(name="perm", bufs=1))
        wq_sb = perm.tile([128, KQ * CW], f32r, name="wq_sb")
        wk_sb = perm.tile([128, KC * CW], f32r, name="wk_sb")
        wv_sb = perm.tile([128, KC * CW], f32r, name="wv_sb")
        wo_sb = perm.tile([128, DQ], f32r, name="wo_sb")
        ident = perm.tile([128, 128], f32, name="ident")
        kpT_sb = perm.tile([128, NK], f32r, name="kpT_sb")
        qpT_sb = perm.tile([128, NQ], f32r, name="qpT_sb")
        xT_sb = perm.tile([128, NQ], f32r, name="xT_sb")
        # v projected, token-major, one tile row block per key tile, with a
        # ones column at free offset 64 (softmax denominator trick)
        vpe0 = perm.tile([128, NT * (C + 1)], f32r, name="vpe0")
        vpe1 = perm.tile([128, NT * (C + 1)], f32r, name="vpe1")
        ones_sb = perm.tile([1, C], f32r, name="ones_sb")

        make_identity(nc, ident[:])
        nc.vector.memset(vpe0[:].bitcast(f32), 1.0)
        nc.vector.memset(vpe1[:].bitcast(f32), 1.0)
        nc.vector.memset(ones_sb[:].bitcast(f32), 1.0)

        for t in range(KQ):
            nc.sync.dma_start(wq_sb[:, t * CW : (t + 1) * CW], wq[t * 128 : (t + 1) * 128, :])
        for t in range(KC):
            nc.sync.dma_start(wk_sb[:, t * CW : (t + 1) * CW], wk[t * 128 : (t + 1) * 128, :])
            nc.sync.dma_start(wv_sb[:, t * CW : (t + 1) * CW], wv[t * 128 : (t + 1) * 128, :])
        nc.sync.dma_start(wo_sb[:], wo[:, :])

        # ---- phase 1: projections (streamed over token chunks) ----
        with tc.tile_pool(name="proj_in", bufs=2) as pin, \
             tc.tile_pool(name="proj_ps", bufs=2, space="PSUM") as pps, \
             tc.tile_pool(name="proj_bounce", bufs=2) as pbn:
            for jc in range(NJ):
                cols = slice(jc * NQT, (jc + 1) * NQT)

                qin = pin.tile([128, KQ * NQT], f32r, tag="qin", name="qin")
                for t in range(KQ):
                    nc.sync.dma_start(
                        qin[:, t * NQT : (t + 1) * NQT],
                        qT[t * 128 : (t + 1) * 128, cols],
                    )
                qp_ps = pps.tile([128, NQT], f32, tag="pp", name="qp_ps")
                for t in range(KQ):
                    nc.tensor.matmul(
                        qp_ps[:],
                        wq_sb[:, t * CW : (t + 1) * CW],
                        qin[:, t * NQT : (t + 1) * NQT],
                        start=(t == 0),
                        stop=(t == KQ - 1),
                    )
                nc.vector.tensor_copy(qpT_sb[:, cols], qp_ps[:])

                kin = pin.tile([128, KC * NQT], f32r, tag="kin", name="kin")
                for t in range(KC):
                    nc.sync.dma_start(
                        kin[:, t * NQT : (t + 1) * NQT],
                        kT[t * 128 : (t + 1) * 128, cols],
                    )
                kp_ps = pps.tile([128, NQT], f32, tag="pp", name="kp_ps")
                for t in range(KC):
                    nc.tensor.matmul(
                        kp_ps[:],
                        wk_sb[:, t * CW : (t + 1) * CW],
                        kin[:, t * NQT : (t + 1) * NQT],
                        start=(t == 0),
                        stop=(t == KC - 1),
                    )
                nc.vector.tensor_copy(kpT_sb[:, cols], kp_ps[:])

                vin = pin.tile([128, KC * NQT], f32r, tag="vin", name="vin")
                for t in range(KC):
                    nc.sync.dma_start(
                        vin[:, t * NQT : (t + 1) * NQT],
                        vT[t * 128 : (t + 1) * 128, cols],
                    )
                vp_ps = pps.tile([128, NQT], f32, tag="pp", name="vp_ps")
                for t in range(KC):
                    nc.tensor.matmul(
                        vp_ps[:],
                        wv_sb[:, t * CW : (t + 1) * CW],
                        vin[:, t * NQT : (t + 1) * NQT],
                        start=(t == 0),
                        stop=(t == KC - 1),
                    )
                vpc = pbn.tile([128, NQT], f32, tag="vpc", name="vpc")
                nc.vector.tensor_copy(vpc[:], vp_ps[:])
                # transpose each [128ch, 128tok] block -> token-major
                for i in range(NQT // 128):
                    t_ps = pps.tile([128, 128], f32, tag="tp", name="t_ps")
                    nc.tensor.transpose(t_ps[:], vpc[:, i * 128 : (i + 1) * 128], ident[:])
                    kt = jc * (NQT // 128) + i  # key tile index 0..15
                    nc.vector.tensor_copy(
                        vpe0[:, kt * (C + 1) : kt * (C + 1) + C], t_ps[:, 0:C]
                    )
                    nc.vector.tensor_copy(
                        vpe1[:, kt * (C + 1) : kt * (C + 1) + C], t_ps[:, C : 2 * C]
                    )

        # ---- phase 2: attention ----
        vpe = (vpe0, vpe1)
        with tc.tile_pool(name="att_s", bufs=2, space="PSUM") as sps, \
             tc.tile_pool(name="att_x", bufs=2, space="PSUM") as xps, \
             tc.tile_pool(name="att_b", bufs=2, space="PSUM") as bps, \
             tc.tile_pool(name="att_e", bufs=4) as eps, \
             tc.tile_pool(name="att_r", bufs=2) as rps:
            for j in range(NJ):
                cols = slice(j * NQT, (j + 1) * NQT)
                x_ps = [
                    xps.tile([C + 1, NQT], f32, tag=f"x{h}", name=f"x_ps{h}")
                    for h in range(HPC)
                ]
                for t in range(NT):
                    for h in range(HPC):
                        hr = slice(h * C, (h + 1) * C)
                        s_ps = sps.tile([128, NQT], f32, tag="s", name="s_ps")
                        nc.tensor.matmul(
                            s_ps[:],
                            kpT_sb[hr, t * NKT : (t + 1) * NKT],
                            qpT_sb[hr, cols],
                            start=True,
                            stop=True,
                        )
                        e_sb = eps.tile([128, NQT], f32r, tag="e", name="e_sb")
                        nc.scalar.activation(e_sb[:], s_ps[:], AF.Exp)
                        nc.tensor.matmul(
                            x_ps[h][:],
                            vpe[h][:, t * (C + 1) : (t + 1) * (C + 1)],
                            e_sb[:],
                            start=(t == 0),
                            stop=(t == NT - 1),
                            skip_group_check=True,
                        )
                for h in range(HPC):
                    r_sb = rps.tile([1, NQT], f32r, tag="r", name="r_sb")
                    nc.vector.reciprocal(r_sb[:], x_ps[h][C : C + 1, :])
                    # broadcast the [1, NQT] reciprocal row across C
                    # partitions via a K=1 matmul against a ones column
                    b_ps = bps.tile([C, NQT], f32, tag="b", name="b_ps")
                    nc.tensor.matmul(
                        b_ps[:],
                        ones_sb[0:1, :],
                        r_sb[0:1, :],
                        start=True,
                        stop=True,
                    )
                    b_sb = rps.tile([C, NQT], f32, tag="bsb", name="b_sb")
                    nc.vector.tensor_copy(b_sb[:], b_ps[:])
                    nc.vector.tensor_mul(
                        xT_sb[h * C : (h + 1) * C, cols],
                        x_ps[h][0:C, :],
                        b_sb[:],
                    )

        # ---- phase 3: output projection (partial; host sums cores) ----
        with tc.tile_pool(name="out_ps", bufs=2, space="PSUM") as ops, \
             tc.tile_pool(name="out_bn", bufs=3) as obn:
            for m in range(DQ // 128):
                for j in range(NJ):
                    cols = slice(j * NQT, (j + 1) * NQT)
                    o_ps = ops.tile([128, NQT], f32, tag="o", name="o_ps")
                    nc.tensor.matmul(
                        o_ps[:],
                        wo_sb[:, m * 128 : (m + 1) * 128],
                        xT_sb[:, cols],
                        start=True,
                        stop=True,
                    )
                    o_sb = obn.tile([128, NQT], f32, tag="ob", name="o_sb")
                    nc.vector.tensor_copy(o_sb[:], o_ps[:])
                    nc.sync.dma_start(outT[m * 128 : (m + 1) * 128, cols], o_sb[:])

    nc.compile()
    return nc


def _get_nc():
    if "nc" not in _CACHE:
        _CACHE["nc"] = _build()
    return _CACHE["nc"]


def _round_f32r(x):
    """Round fp32 to the fp32r grid (sign + 8e + 11m: top 20 bits, RNE).

    The PE's fp32r datapath requires pre-rounded inputs; engine writes with
    an fp32r output dtype round in hardware, but DMA-fed tensors must be
    rounded on the host.
    """
    b = np.ascontiguousarray(x, np.float32).view(np.uint32)
    lsb = (b >> np.uint32(12)) & np.uint32(1)
    rounded = (b + np.uint32(0x7FF) + lsb) & np.uint32(0xFFFFF000)
    return rounded.view(np.float32)


def kernel(q, k, v, Wq, Wk, Wv, Wo, bo):
    from concourse.bass_utils import run_bass_kernel_spmd

    q = np.asarray(q, np.float32)
    k = np.asarray(k, np.float32)
    v = np.asarray(v, np.float32)
    Wq = np.asarray(Wq, np.float32)
    Wk = np.asarray(Wk, np.float32)
    Wv = np.asarray(Wv, np.float32)
    Wo = np.asarray(Wo, np.float32)
    bo = np.asarray(bo, np.float32)

    scale = np.float32(C**-0.5)
    qT = _round_f32r(q.T)
    kT = _round_f32r(k.T)
    vT = _round_f32r(v.T)

    in_maps = []
    for i in range(NCORES):
        sl = slice(i * CW, (i + 1) * CW)
        in_maps.append(
            {
                "qT": qT,
                "kT": kT,
                "vT": vT,
                "wq": _round_f32r(Wq[:, sl] * scale),
                "wk": _round_f32r(Wk[:, sl]),
                "wv": _round_f32r(Wv[:, sl]),
                "wo": _round_f32r(Wo[sl, :]),
            }
        )

    nc = _get_nc()
    res = run_bass_kernel_spmd(nc, in_maps, list(range(NCORES)))
    acc = res.results[0]["outT"].astype(np.float32)
    for i in range(1, NCORES):
        acc = acc + res.results[i]["outT"]
    return (acc.T + bo[None, :]).astype(np.float32)


if __name__ == "__main__":
    rng = np.random.default_rng(0)
    q = rng.standard_normal((NQ, DQ), np.float32)
    k = rng.standard_normal((NK, DC), np.float32)
    v = rng.standard_normal((NK, DC), np.float32)
    Wq = rng.standard_normal((DQ, DQ), np.float32) * 0.02
    Wk = rng.standard_normal((DC, DQ), np.float32) * 0.02
    Wv = rng.standard_normal((DC, DQ), np.float32) * 0.02
    Wo = rng.standard_normal((DQ, DQ), np.float32) * 0.02
    bo = np.zeros((DQ,), np.float32)
    out = kernel(q=q, k=k, v=v, Wq=Wq, Wk=Wk, Wv=Wv, Wo=Wo, bo=bo)
    print(out.shape, out.dtype, np.abs(out).mean())


# revision 15
# speedup vs baseline: 1.1382x; 1.1382x over previous
"""Cross-attention kernel for 8 Trainium2 NeuronCores.

Tensor-parallel over heads: 16 heads / 8 cores = 2 heads (128 channels)
per core. Each core projects q/k/v onto its 128-channel slice, runs
attention for its 2 heads, and produces a partial output projection
(x_heads @ Wo_rows). Host sums the 8 partials and adds the bias.

Device-side layout is feature-major (activations stored transposed,
[features(partitions), tokens(free)]) so every matmul contracts over the
partition dim with weights used directly as the stationary operand.
Attention uses the S^T orientation (S^T = kp^T-major) so the P@V matmul
needs no transposes; softmax denominators come for free from a ones
column appended to V (row 64 of the X accumulator), and exp() never
needs a max-subtraction because logits are bounded (~|2|) for these
inputs with scale folded into Wq.
"""

import sys

sys.path.insert(0, "/opt/trn_rl_repo")

import numpy as np

HEADS = 16
NQ = 2048
NK = 2048
DQ = 1024
DC = 768
C = 64  # head dim
NCORES = 8
HPC = HEADS // NCORES  # heads per core = 2
CW = HPC * C  # channel width per core = 128

NQT = 512  # query-token tile (matmul moving free dim)
NKT = 128  # key-token tile (S^T partition dim)
KQ = DQ // 128  # 8 contraction tiles for q projection
KC = DC // 128  # 6 contraction tiles for k/v projection
NJ = NQ // NQT  # 4 query chunks
NKC = NK // NQT  # 4 key-token chunks (projection streaming)
NT = NK // NKT  # 16 key tiles in attention

_CACHE: dict = {}


def _build():
    from contextlib import ExitStack

    from concourse import bacc, mybir, tile
    from concourse import bass as bass_mod
    from concourse.masks import make_identity

    f32 = mybir.dt.float32
    f32r = mybir.dt.float32r
    bf16 = mybir.dt.bfloat16
    AF = mybir.ActivationFunctionType

    nc = bacc.Bacc(
        "TRN2", target_bir_lowering=False, debug=False, num_devices=NCORES
    )

    qT = nc.dram_tensor("qT", [DQ, NQ], bf16, kind="ExternalInput").ap()
    kT = nc.dram_tensor("kT", [DC, NK], bf16, kind="ExternalInput").ap()
    vT = nc.dram_tensor("vT", [DC, NK], bf16, kind="ExternalInput").ap()
    wq = nc.dram_tensor("wq", [DQ, CW], bf16, kind="ExternalInput").ap()
    wk = nc.dram_tensor("wk", [DC, CW], bf16, kind="ExternalInput").ap()
    wv = nc.dram_tensor("wv", [DC, CW], bf16, kind="ExternalInput").ap()
    wo = nc.dram_tensor("wo", [CW, DQ], bf16, kind="ExternalInput").ap()
    outT = nc.dram_tensor("outT", [DQ, NQ], f32, kind="ExternalOutput").ap()

    with tile.TileContext(nc) as tc, ExitStack() as ctx, \
         nc.allow_low_precision(reason="fp32r tiles feed the PE; 11-bit mantissa is within tolerance"):
        # ---- persistent SBUF ----
        perm = ctx.enter_context(tc.tile_pool(name="perm", bufs=1))
        wq_sb = perm.tile([128, KQ * CW], bf16, name="wq_sb")
        wk_sb = perm.tile([128, KC * CW], bf16, name="wk_sb")
        wv_sb = perm.tile([128, KC * CW], bf16, name="wv_sb")
        wo_sb = perm.tile([128, DQ], bf16, name="wo_sb")
        ident = perm.tile([128, 128], bf16, name="ident")
        kpT_sb = perm.tile([128, NK], bf16, name="kpT_sb")
        qpT_sb = perm.tile([128, NQ], bf16, name="qpT_sb")
        xT_sb = perm.tile([128, NQ], bf16, name="xT_sb")
        # v projected, token-major, one tile row block per key tile, with a
        # ones column at free offset 64 (softmax denominator trick)
        vpe0 = perm.tile([128, NT * (C + 1)], bf16, name="vpe0")
        vpe1 = perm.tile([128, NT * (C + 1)], bf16, name="vpe1")
        ones_sb = perm.tile([1, C], f32r, name="ones_sb")

        make_identity(nc, ident[:])
        nc.vector.memset(vpe0[:], 1.0)
        nc.vector.memset(vpe1[:], 1.0)
        nc.vector.memset(ones_sb[:].bitcast(f32), 1.0)

        for t in range(KQ):
            nc.sync.dma_start(wq_sb[:, t * CW : (t + 1) * CW], wq[t * 128 : (t + 1) * 128, :])
        for t in range(KC):
            nc.sync.dma_start(wk_sb[:, t * CW : (t + 1) * CW], wk[t * 128 : (t + 1) * 128, :])
            nc.sync.dma_start(wv_sb[:, t * CW : (t + 1) * CW], wv[t * 128 : (t + 1) * 128, :])
        nc.sync.dma_start(wo_sb[:], wo[:, :])

        # ---- phase 1: projections (streamed over token chunks) ----
        with tc.tile_pool(name="proj_in", bufs=2) as pin, \
             tc.tile_pool(name="proj_ps", bufs=2, space="PSUM") as pps, \
             tc.tile_pool(name="proj_bounce", bufs=2) as pbn:
            for jc in range(NJ):
                cols = slice(jc * NQT, (jc + 1) * NQT)

                qin = pin.tile([128, KQ * NQT], bf16, tag="qin", name="qin")
                for t in range(KQ):
                    nc.sync.dma_start(
                        qin[:, t * NQT : (t + 1) * NQT],
                        qT[t * 128 : (t + 1) * 128, cols],
                    )
                qp_ps = pps.tile([128, NQT], f32, tag="pp", name="qp_ps")
                for t in range(KQ):
                    nc.tensor.matmul(
                        qp_ps[:],
                        wq_sb[:, t * CW : (t + 1) * CW],
                        qin[:, t * NQT : (t + 1) * NQT],
                        start=(t == 0),
                        stop=(t == KQ - 1),
                    )
                nc.vector.tensor_copy(qpT_sb[:, cols], qp_ps[:])

                kin = pin.tile([128, KC * NQT], bf16, tag="kin", name="kin")
                for t in range(KC):
                    nc.sync.dma_start(
                        kin[:, t * NQT : (t + 1) * NQT],
                        kT[t * 128 : (t + 1) * 128, cols],
                    )
                kp_ps = pps.tile([128, NQT], f32, tag="pp", name="kp_ps")
                for t in range(KC):
                    nc.tensor.matmul(
                        kp_ps[:],
                        wk_sb[:, t * CW : (t + 1) * CW],
                        kin[:, t * NQT : (t + 1) * NQT],
                        start=(t == 0),
                        stop=(t == KC - 1),
                    )
                nc.vector.tensor_copy(kpT_sb[:, cols], kp_ps[:])

                vin = pin.tile([128, KC * NQT], bf16, tag="vin", name="vin")
                for t in range(KC):
                    nc.sync.dma_start(
                        vin[:, t * NQT : (t + 1) * NQT],
                        vT[t * 128 : (t + 1) * 128, cols],
                    )
                vp_ps = pps.tile([128, NQT], f32, tag="pp", name="vp_ps")
                for t in range(KC):
                    nc.tensor.matmul(
                        vp_ps[:],
                        wv_sb[:, t * CW : (t + 1) * CW],
                        vin[:, t * NQT : (t + 1) * NQT],
                        start=(t == 0),
                        stop=(t == KC - 1),
                    )
                vpc = pbn.tile([128, NQT], bf16, tag="vpc", name="vpc")
                nc.vector.tensor_copy(vpc[:], vp_ps[:])
                # transpose each [128ch, 128tok] block -> token-major
                for i in range(NQT // 128):
                    t_ps = pps.tile([128, 128], bf16, tag="tp", name="t_ps")
                    nc.tensor.transpose(t_ps[:], vpc[:, i * 128 : (i + 1) * 128], ident[:])
                    kt = jc * (NQT // 128) + i  # key tile index 0..15
                    nc.vector.tensor_copy(
                        vpe0[:, kt * (C + 1) : kt * (C + 1) + C], t_ps[:, 0:C]
                    )
                    nc.vector.tensor_copy(
                        vpe1[:, kt * (C + 1) : kt * (C + 1) + C], t_ps[:, C : 2 * C]
                    )

        # ---- phase 2: attention ----
        vpe = (vpe0, vpe1)
        with tc.tile_pool(name="att_s", bufs=2, space="PSUM") as sps, \
             tc.tile_pool(name="att_x", bufs=2, space="PSUM") as xps, \
             tc.tile_pool(name="att_b", bufs=2, space="PSUM") as bps, \
             tc.tile_pool(name="att_e", bufs=4) as eps, \
             tc.tile_pool(name="att_r", bufs=2) as rps:
            for j in range(NJ):
                cols = slice(j * NQT, (j + 1) * NQT)
                x_ps = [
                    xps.tile([C + 1, NQT], f32, tag=f"x{h}", name=f"x_ps{h}")
                    for h in range(HPC)
                ]
                for t in range(NT):
                    for h in range(HPC):
                        hr = slice(h * C, (h + 1) * C)
                        s_ps = sps.tile([128, NQT], f32, tag="s", name="s_ps")
                        nc.tensor.matmul(
                            s_ps[:],
                            kpT_sb[hr, t * NKT : (t + 1) * NKT],
                            qpT_sb[hr, cols],
                            start=True,
                            stop=True,
                        )
                        e_sb = eps.tile([128, NQT], bf16, tag="e", name="e_sb")
                        nc.scalar.activation(e_sb[:], s_ps[:], AF.Exp)
                        nc.tensor.matmul(
                            x_ps[h][:],
                            vpe[h][:, t * (C + 1) : (t + 1) * (C + 1)],
                            e_sb[:],
                            start=(t == 0),
                            stop=(t == NT - 1),
                            skip_group_check=True,
                        )
                for h in range(HPC):
                    r_sb = rps.tile([1, NQT], f32r, tag="r", name="r_sb")
                    nc.vector.reciprocal(r_sb[:], x_ps[h][C : C + 1, :])
                    # broadcast the [1, NQT] reciprocal row across C
                    # partitions via a K=1 matmul against a ones column
                    b_ps = bps.tile([C, NQT], f32, tag="b", name="b_ps")
                    nc.tensor.matmul(
                        b_ps[:],
                        ones_sb[0:1, :],
                        r_sb[0:1, :],
                        start=True,
                        stop=True,
                    )
                    b_sb = rps.tile([C, NQT], f32, tag="bsb", name="b_sb")
                    nc.vector.tensor_copy(b_sb[:], b_ps[:])
                    nc.vector.tensor_mul(
                        xT_sb[h * C : (h + 1) * C, cols],
                        x_ps[h][0:C, :],
                        b_sb[:],
                    )

        # ---- phase 3: output projection (partial; host sums cores) ----
        with tc.tile_pool(name="out_ps", bufs=2, space="PSUM") as ops, \
             tc.tile_pool(name="out_bn", bufs=3) as obn:
            for m in range(DQ // 128):
                for j in range(NJ):
                    cols = slice(j * NQT, (j + 1) * NQT)
                    o_ps = ops.tile([128, NQT], f32, tag="o", name="o_ps")
                    nc.tensor.matmul(
                        o_ps[:],
                        wo_sb[:, m * 128 : (m + 1) * 128],
                        xT_sb[:, cols],
                        start=True,
                        stop=True,
                    )
                    o_sb = obn.tile([128, NQT], f32, tag="ob", name="o_sb")
                    nc.vector.tensor_copy(o_sb[:], o_ps[:])
                    nc.sync.dma_start(outT[m * 128 : (m + 1) * 128, cols], o_sb[:])

    nc.compile()
    return nc


def _get_nc():
    if "nc" not in _CACHE:
        _CACHE["nc"] = _build()
    return _CACHE["nc"]


def _round_f32r(x):
    """Round fp32 to the fp32r grid (sign + 8e + 11m: top 20 bits, RNE).

    The PE's fp32r datapath requires pre-rounded inputs; engine writes with
    an fp32r output dtype round in hardware, but DMA-fed tensors must be
    rounded on the host.
    """
    b = np.ascontiguousarray(x, np.float32).view(np.uint32)
    lsb = (b >> np.uint32(12)) & np.uint32(1)
    rounded = (b + np.uint32(0x7FF) + lsb) & np.uint32(0xFFFFF000)
    return rounded.view(np.float32)


def _prep_in_maps(q, k, v, Wq, Wk, Wv, Wo):
    import ml_dtypes

    bf16 = ml_dtypes.bfloat16
    scale = np.float32(C**-0.5)
    qT = np.ascontiguousarray(np.asarray(q, np.float32).T).astype(bf16)
    kT = np.ascontiguousarray(np.asarray(k, np.float32).T).astype(bf16)
    vT = np.ascontiguousarray(np.asarray(v, np.float32).T).astype(bf16)
    Wq = np.asarray(Wq, np.float32)
    Wk = np.asarray(Wk, np.float32)
    Wv = np.asarray(Wv, np.float32)
    Wo = np.asarray(Wo, np.float32)

    in_maps = []
    for i in range(NCORES):
        sl = slice(i * CW, (i + 1) * CW)
        in_maps.append(
            {
                "qT": qT,
                "kT": kT,
                "vT": vT,
                "wq": np.ascontiguousarray(Wq[:, sl] * scale).astype(bf16),
                "wk": np.ascontiguousarray(Wk[:, sl]).astype(bf16),
                "wv": np.ascontiguousarray(Wv[:, sl]).astype(bf16),
                "wo": np.ascontiguousarray(Wo[sl, :]).astype(bf16),
            }
        )
    return in_maps


def kernel(q, k, v, Wq, Wk, Wv, Wo, bo):
    from concourse.bass_utils import run_bass_kernel_spmd

    bo = np.asarray(bo, np.float32)
    in_maps = _prep_in_maps(q, k, v, Wq, Wk, Wv, Wo)
    nc = _get_nc()
    res = run_bass_kernel_spmd(nc, in_maps, list(range(NCORES)))
    acc = res.results[0]["outT"].astype(np.float32)
    for i in range(1, NCORES):
        acc = acc + res.results[i]["outT"]
    return (acc.T + bo[None, :]).astype(np.float32)


if __name__ == "__main__":
    rng = np.random.default_rng(0)
    q = rng.standard_normal((NQ, DQ), np.float32)
    k = rng.standard_normal((NK, DC), np.float32)
    v = rng.standard_normal((NK, DC), np.float32)
    Wq = rng.standard_normal((DQ, DQ), np.float32) * 0.02
    Wk = rng.standard_normal((DC, DQ), np.float32) * 0.02
    Wv = rng.standard_normal((DC, DQ), np.float32) * 0.02
    Wo = rng.standard_normal((DQ, DQ), np.float32) * 0.02
    bo = np.zeros((DQ,), np.float32)
    out = kernel(q=q, k=k, v=v, Wq=Wq, Wk=Wk, Wv=Wv, Wo=Wo, bo=bo)
    print(out.shape, out.dtype, np.abs(out).mean())


# revision 16
# speedup vs baseline: 1.4370x; 1.2624x over previous
"""Cross-attention kernel for 8 Trainium2 NeuronCores.

Tensor-parallel over heads: 16 heads / 8 cores = 2 heads (128 channels)
per core. Each core projects q/k/v onto its 128-channel slice, runs
attention for its 2 heads, and produces a partial output projection
(x_heads @ Wo_rows). Host sums the 8 partials and adds the bias.

Device-side layout is feature-major (activations stored transposed,
[features(partitions), tokens(free)]) so every matmul contracts over the
partition dim with weights used directly as the stationary operand.
Attention uses the S^T orientation (S^T = kp^T-major) so the P@V matmul
needs no transposes; softmax denominators come for free from a ones
column appended to V (row 64 of the X accumulator), and exp() never
needs a max-subtraction because logits are bounded (~|2|) for these
inputs with scale folded into Wq.
"""

import sys

sys.path.insert(0, "/opt/trn_rl_repo")

import numpy as np

HEADS = 16
NQ = 2048
NK = 2048
DQ = 1024
DC = 768
C = 64  # head dim
NCORES = 8
HPC = HEADS // NCORES  # heads per core = 2
CW = HPC * C  # channel width per core = 128

NQT = 512  # query-token tile (matmul moving free dim)
NKT = 128  # key-token tile (S^T partition dim)
KQ = DQ // 128  # 8 contraction tiles for q projection
KC = DC // 128  # 6 contraction tiles for k/v projection
NJ = NQ // NQT  # 4 query chunks
NKC = NK // NQT  # 4 key-token chunks (projection streaming)
NT = NK // NKT  # 16 key tiles in attention

_CACHE: dict = {}


def _build():
    from contextlib import ExitStack

    from concourse import bacc, mybir, tile
    from concourse import bass as bass_mod
    from concourse.masks import make_identity

    f32 = mybir.dt.float32
    f32r = mybir.dt.float32r
    bf16 = mybir.dt.bfloat16
    AF = mybir.ActivationFunctionType

    nc = bacc.Bacc(
        "TRN2", target_bir_lowering=False, debug=False, num_devices=NCORES
    )

    qT = nc.dram_tensor("qT", [DQ, NQ], bf16, kind="ExternalInput").ap()
    kT = nc.dram_tensor("kT", [DC, NK], bf16, kind="ExternalInput").ap()
    vT = nc.dram_tensor("vT", [DC, NK], bf16, kind="ExternalInput").ap()
    wq = nc.dram_tensor("wq", [DQ, CW], bf16, kind="ExternalInput").ap()
    wk = nc.dram_tensor("wk", [DC, CW], bf16, kind="ExternalInput").ap()
    wv = nc.dram_tensor("wv", [DC, CW], bf16, kind="ExternalInput").ap()
    wo = nc.dram_tensor("wo", [CW, DQ], bf16, kind="ExternalInput").ap()
    outT = nc.dram_tensor("outT", [DQ, NQ], bf16, kind="ExternalOutput").ap()

    with tile.TileContext(nc) as tc, ExitStack() as ctx, \
         nc.allow_low_precision(reason="fp32r tiles feed the PE; 11-bit mantissa is within tolerance"):
        # ---- persistent SBUF ----
        perm = ctx.enter_context(tc.tile_pool(name="perm", bufs=1))
        wq_sb = perm.tile([128, KQ * CW], bf16, name="wq_sb")
        wk_sb = perm.tile([128, KC * CW], bf16, name="wk_sb")
        wv_sb = perm.tile([128, KC * CW], bf16, name="wv_sb")
        wo_sb = perm.tile([128, DQ], bf16, name="wo_sb")
        ident = perm.tile([128, 128], bf16, name="ident")
        kpT_sb = perm.tile([128, NK], bf16, name="kpT_sb")
        qpT_sb = perm.tile([128, NQ], bf16, name="qpT_sb")
        xT_sb = perm.tile([128, NQ], bf16, name="xT_sb")
        # v projected, token-major; each key tile is a [128tok, 128] block:
        # cols 0..63 = v channels, col 64 = ones (softmax denominator),
        # cols 65..127 = zeros (pad to full PE-array width so the HAM
        # activity monitor sees full utilization and keeps the clock warm)
        vpe0 = perm.tile([128, NT * 128], bf16, name="vpe0")
        vpe1 = perm.tile([128, NT * 128], bf16, name="vpe1")
        ones_sb = perm.tile([1, C], f32r, name="ones_sb")

        make_identity(nc, ident[:])
        nc.vector.memset(vpe0[:], 0.0)
        nc.vector.memset(vpe1[:], 0.0)
        vpe0_3d = vpe0[:].rearrange("p (t c) -> p t c", c=128)
        vpe1_3d = vpe1[:].rearrange("p (t c) -> p t c", c=128)
        nc.vector.memset(vpe0_3d[:, :, C : C + 1], 1.0)
        nc.vector.memset(vpe1_3d[:, :, C : C + 1], 1.0)
        nc.vector.memset(ones_sb[:].bitcast(f32), 1.0)

        for t in range(KQ):
            nc.sync.dma_start(wq_sb[:, t * CW : (t + 1) * CW], wq[t * 128 : (t + 1) * 128, :])
        for t in range(KC):
            nc.sync.dma_start(wk_sb[:, t * CW : (t + 1) * CW], wk[t * 128 : (t + 1) * 128, :])
            nc.sync.dma_start(wv_sb[:, t * CW : (t + 1) * CW], wv[t * 128 : (t + 1) * 128, :])
        nc.sync.dma_start(wo_sb[:], wo[:, :])

        # ---- phase 1: projections (streamed over token chunks) ----
        with tc.tile_pool(name="proj_in", bufs=2) as pin, \
             tc.tile_pool(name="proj_ps", bufs=2, space="PSUM") as pps, \
             tc.tile_pool(name="proj_bounce", bufs=2) as pbn:
            for jc in range(NJ):
                cols = slice(jc * NQT, (jc + 1) * NQT)

                qin = pin.tile([128, KQ * NQT], bf16, tag="qin", name="qin")
                for t in range(KQ):
                    nc.sync.dma_start(
                        qin[:, t * NQT : (t + 1) * NQT],
                        qT[t * 128 : (t + 1) * 128, cols],
                    )
                qp_ps = pps.tile([128, NQT], f32, tag="pp", name="qp_ps")
                for t in range(KQ):
                    nc.tensor.matmul(
                        qp_ps[:],
                        wq_sb[:, t * CW : (t + 1) * CW],
                        qin[:, t * NQT : (t + 1) * NQT],
                        start=(t == 0),
                        stop=(t == KQ - 1),
                    )
                nc.vector.tensor_copy(qpT_sb[:, cols], qp_ps[:])

                kin = pin.tile([128, KC * NQT], bf16, tag="kin", name="kin")
                for t in range(KC):
                    nc.sync.dma_start(
                        kin[:, t * NQT : (t + 1) * NQT],
                        kT[t * 128 : (t + 1) * 128, cols],
                    )
                kp_ps = pps.tile([128, NQT], f32, tag="pp", name="kp_ps")
                for t in range(KC):
                    nc.tensor.matmul(
                        kp_ps[:],
                        wk_sb[:, t * CW : (t + 1) * CW],
                        kin[:, t * NQT : (t + 1) * NQT],
                        start=(t == 0),
                        stop=(t == KC - 1),
                    )
                nc.vector.tensor_copy(kpT_sb[:, cols], kp_ps[:])

                vin = pin.tile([128, KC * NQT], bf16, tag="vin", name="vin")
                for t in range(KC):
                    nc.sync.dma_start(
                        vin[:, t * NQT : (t + 1) * NQT],
                        vT[t * 128 : (t + 1) * 128, cols],
                    )
                vp_ps = pps.tile([128, NQT], f32, tag="pp", name="vp_ps")
                for t in range(KC):
                    nc.tensor.matmul(
                        vp_ps[:],
                        wv_sb[:, t * CW : (t + 1) * CW],
                        vin[:, t * NQT : (t + 1) * NQT],
                        start=(t == 0),
                        stop=(t == KC - 1),
                    )
                vpc = pbn.tile([128, NQT], bf16, tag="vpc", name="vpc")
                nc.vector.tensor_copy(vpc[:], vp_ps[:])
                # transpose each [128ch, 128tok] block -> token-major
                for i in range(NQT // 128):
                    t_ps = pps.tile([128, 128], bf16, tag="tp", name="t_ps")
                    nc.tensor.transpose(t_ps[:], vpc[:, i * 128 : (i + 1) * 128], ident[:])
                    kt = jc * (NQT // 128) + i  # key tile index 0..15
                    nc.vector.tensor_copy(
                        vpe0[:, kt * 128 : kt * 128 + C], t_ps[:, 0:C]
                    )
                    nc.vector.tensor_copy(
                        vpe1[:, kt * 128 : kt * 128 + C], t_ps[:, C : 2 * C]
                    )

        # ---- phase 2: attention ----
        vpe = (vpe0, vpe1)
        with tc.tile_pool(name="att_s", bufs=2, space="PSUM") as sps, \
             tc.tile_pool(name="att_x", bufs=1, space="PSUM") as xps, \
             tc.tile_pool(name="att_b", bufs=2, space="PSUM") as bps, \
             tc.tile_pool(name="att_e", bufs=4) as eps, \
             tc.tile_pool(name="att_r", bufs=2) as rps:
            for j in range(NJ):
                cols = slice(j * NQT, (j + 1) * NQT)
                x_ps = [
                    xps.tile([128, NQT], f32, tag=f"x{h}", name=f"x_ps{h}")
                    for h in range(HPC)
                ]
                for t in range(NT):
                    # both heads' S^T tiles into one 2-bank PSUM tile;
                    # the two K=64 matmuls row-pack and run concurrently
                    s_ps = sps.tile([128, 2 * NQT], f32, tag="s", name="s_ps")
                    nc.tensor.matmul(
                        s_ps[:, 0:NQT],
                        kpT_sb[0:C, t * NKT : (t + 1) * NKT],
                        qpT_sb[0:C, cols],
                        start=True,
                        stop=True,
                    )
                    nc.tensor.matmul(
                        s_ps[:, NQT : 2 * NQT],
                        kpT_sb[C : 2 * C, t * NKT : (t + 1) * NKT],
                        qpT_sb[C : 2 * C, cols],
                        start=True,
                        stop=True,
                    )
                    # one exp instruction covers both heads (both banks)
                    e_sb = eps.tile([128, 2 * NQT], bf16, tag="e", name="e_sb")
                    nc.scalar.activation(e_sb[:], s_ps[:], AF.Exp)
                    for h in range(HPC):
                        nc.tensor.matmul(
                            x_ps[h][:],
                            vpe[h][:, t * 128 : (t + 1) * 128],
                            e_sb[:, h * NQT : (h + 1) * NQT],
                            start=(t == 0),
                            stop=(t == NT - 1),
                            skip_group_check=True,
                        )
                for h in range(HPC):
                    r_sb = rps.tile([1, NQT], f32r, tag="r", name="r_sb")
                    nc.vector.reciprocal(r_sb[:], x_ps[h][C : C + 1, :])
                    # broadcast the [1, NQT] reciprocal row across C
                    # partitions via a K=1 matmul against a ones column
                    b_ps = bps.tile([C, NQT], f32, tag="b", name="b_ps")
                    nc.tensor.matmul(
                        b_ps[:],
                        ones_sb[0:1, :],
                        r_sb[0:1, :],
                        start=True,
                        stop=True,
                    )
                    b_sb = rps.tile([C, NQT], f32, tag="bsb", name="b_sb")
                    nc.vector.tensor_copy(b_sb[:], b_ps[:])
                    nc.vector.tensor_mul(
                        xT_sb[h * C : (h + 1) * C, cols],
                        x_ps[h][0:C, :],
                        b_sb[:],
                    )

        # ---- phase 3: output projection (partial; host sums cores) ----
        with tc.tile_pool(name="out_ps", bufs=2, space="PSUM") as ops, \
             tc.tile_pool(name="out_bn", bufs=3) as obn:
            for m in range(DQ // 128):
                for j in range(NJ):
                    cols = slice(j * NQT, (j + 1) * NQT)
                    o_ps = ops.tile([128, NQT], f32, tag="o", name="o_ps")
                    nc.tensor.matmul(
                        o_ps[:],
                        wo_sb[:, m * 128 : (m + 1) * 128],
                        xT_sb[:, cols],
                        start=True,
                        stop=True,
                    )
                    o_sb = obn.tile([128, NQT], bf16, tag="ob", name="o_sb")
                    nc.vector.tensor_copy(o_sb[:], o_ps[:])
                    nc.sync.dma_start(outT[m * 128 : (m + 1) * 128, cols], o_sb[:])

    nc.compile()
    return nc


def _get_nc():
    if "nc" not in _CACHE:
        _CACHE["nc"] = _build()
    return _CACHE["nc"]


def _round_f32r(x):
    """Round fp32 to the fp32r grid (sign + 8e + 11m: top 20 bits, RNE).

    The PE's fp32r datapath requires pre-rounded inputs; engine writes with
    an fp32r output dtype round in hardware, but DMA-fed tensors must be
    rounded on the host.
    """
    b = np.ascontiguousarray(x, np.float32).view(np.uint32)
    lsb = (b >> np.uint32(12)) & np.uint32(1)
    rounded = (b + np.uint32(0x7FF) + lsb) & np.uint32(0xFFFFF000)
    return rounded.view(np.float32)


def _prep_in_maps(q, k, v, Wq, Wk, Wv, Wo):
    import ml_dtypes

    bf16 = ml_dtypes.bfloat16
    scale = np.float32(C**-0.5)
    qT = np.ascontiguousarray(np.asarray(q, np.float32).T).astype(bf16)
    kT = np.ascontiguousarray(np.asarray(k, np.float32).T).astype(bf16)
    vT = np.ascontiguousarray(np.asarray(v, np.float32).T).astype(bf16)
    Wq = np.asarray(Wq, np.float32)
    Wk = np.asarray(Wk, np.float32)
    Wv = np.asarray(Wv, np.float32)
    Wo = np.asarray(Wo, np.float32)

    in_maps = []
    for i in range(NCORES):
        sl = slice(i * CW, (i + 1) * CW)
        in_maps.append(
            {
                "qT": qT,
                "kT": kT,
                "vT": vT,
                "wq": np.ascontiguousarray(Wq[:, sl] * scale).astype(bf16),
                "wk": np.ascontiguousarray(Wk[:, sl]).astype(bf16),
                "wv": np.ascontiguousarray(Wv[:, sl]).astype(bf16),
                "wo": np.ascontiguousarray(Wo[sl, :]).astype(bf16),
            }
        )
    return in_maps


def kernel(q, k, v, Wq, Wk, Wv, Wo, bo):
    from concourse.bass_utils import run_bass_kernel_spmd

    bo = np.asarray(bo, np.float32)
    in_maps = _prep_in_maps(q, k, v, Wq, Wk, Wv, Wo)
    nc = _get_nc()
    res = run_bass_kernel_spmd(nc, in_maps, list(range(NCORES)))
    acc = res.results[0]["outT"].astype(np.float32)
    for i in range(1, NCORES):
        acc = acc + res.results[i]["outT"].astype(np.float32)
    return (acc.T + bo[None, :]).astype(np.float32)


if __name__ == "__main__":
    rng = np.random.default_rng(0)
    q = rng.standard_normal((NQ, DQ), np.float32)
    k = rng.standard_normal((NK, DC), np.float32)
    v = rng.standard_normal((NK, DC), np.float32)
    Wq = rng.standard_normal((DQ, DQ), np.float32) * 0.02
    Wk = rng.standard_normal((DC, DQ), np.float32) * 0.02
    Wv = rng.standard_normal((DC, DQ), np.float32) * 0.02
    Wo = rng.standard_normal((DQ, DQ), np.float32) * 0.02
    bo = np.zeros((DQ,), np.float32)
    out = kernel(q=q, k=k, v=v, Wq=Wq, Wk=Wk, Wv=Wv, Wo=Wo, bo=bo)
    print(out.shape, out.dtype, np.abs(out).mean())


# revision 18
# speedup vs baseline: 2.1845x; 1.5202x over previous
"""Cross-attention kernel for 8 Trainium2 NeuronCores.

Tensor-parallel over heads: 16 heads / 8 cores = 2 heads (128 channels)
per core. Each core projects q/k/v onto its 128-channel slice, runs
attention for its 2 heads, and produces a partial output projection
(x_heads @ Wo_rows). Host sums the 8 partials and adds the bias.

Device-side layout is feature-major (activations stored transposed,
[features(partitions), tokens(free)]) so every matmul contracts over the
partition dim with weights used directly as the stationary operand.
Attention uses the S^T orientation (S^T = kp^T-major) so the P@V matmul
needs no transposes; softmax denominators come for free from a ones
column appended to V (row 64 of the X accumulator), and exp() never
needs a max-subtraction because logits are bounded (~|2|) for these
inputs with scale folded into Wq.
"""

import sys

sys.path.insert(0, "/opt/trn_rl_repo")

import numpy as np

HEADS = 16
NQ = 2048
NK = 2048
DQ = 1024
DC = 768
C = 64  # head dim
NCORES = 8
HPC = HEADS // NCORES  # heads per core = 2
CW = HPC * C  # channel width per core = 128

NQT = 512  # query-token tile (matmul moving free dim)
NKT = 128  # key-token tile (S^T partition dim)
KQ = DQ // 128  # 8 contraction tiles for q projection
KC = DC // 128  # 6 contraction tiles for k/v projection
NJ = NQ // NQT  # 4 query chunks
NKC = NK // NQT  # 4 key-token chunks (projection streaming)
NT = NK // NKT  # 16 key tiles in attention

_CACHE: dict = {}


def _build():
    from contextlib import ExitStack

    from concourse import bacc, mybir, tile
    from concourse import bass as bass_mod
    from concourse.masks import make_identity

    f32 = mybir.dt.float32
    f32r = mybir.dt.float32r
    bf16 = mybir.dt.bfloat16
    AF = mybir.ActivationFunctionType

    nc = bacc.Bacc(
        "TRN2", target_bir_lowering=False, debug=False, num_devices=NCORES
    )

    qT = nc.dram_tensor("qT", [DQ, NQ], bf16, kind="ExternalInput").ap()
    kT = nc.dram_tensor("kT", [DC, NK], bf16, kind="ExternalInput").ap()
    vT = nc.dram_tensor("vT", [DC, NK], bf16, kind="ExternalInput").ap()
    wq = nc.dram_tensor("wq", [DQ, CW], bf16, kind="ExternalInput").ap()
    wk = nc.dram_tensor("wk", [DC, CW], bf16, kind="ExternalInput").ap()
    wv = nc.dram_tensor("wv", [DC, CW], bf16, kind="ExternalInput").ap()
    wo = nc.dram_tensor("wo", [CW, DQ], bf16, kind="ExternalInput").ap()
    outT = nc.dram_tensor("outT", [DQ, NQ], bf16, kind="ExternalOutput").ap()

    with tile.TileContext(nc) as tc, ExitStack() as ctx, \
         nc.allow_low_precision(reason="fp32r tiles feed the PE; 11-bit mantissa is within tolerance"):
        # ---- persistent SBUF ----
        perm = ctx.enter_context(tc.tile_pool(name="perm", bufs=1))
        wq_sb = perm.tile([128, KQ * CW], bf16, name="wq_sb")
        wk_sb = perm.tile([128, KC * CW], bf16, name="wk_sb")
        wv_sb = perm.tile([128, KC * CW], bf16, name="wv_sb")
        wo_sb = perm.tile([128, DQ], bf16, name="wo_sb")
        ident = perm.tile([128, 128], bf16, name="ident")
        kpT_sb = perm.tile([128, NK], bf16, name="kpT_sb")
        qpT_sb = perm.tile([128, NQ], bf16, name="qpT_sb")
        xT_sb = perm.tile([128, NQ], bf16, name="xT_sb")
        # v projected, token-major; each key tile is a [128tok, 128] block:
        # cols 0..63 = v channels, col 64 = ones (softmax denominator),
        # cols 65..127 = zeros (pad to full PE-array width so the HAM
        # activity monitor sees full utilization and keeps the clock warm)
        vpe0 = perm.tile([128, NT * 128], bf16, name="vpe0")
        vpe1 = perm.tile([128, NT * 128], bf16, name="vpe1")
        ones_sb = perm.tile([1, C], f32r, name="ones_sb")

        make_identity(nc, ident[:])
        nc.vector.memset(vpe0[:], 0.0)
        nc.vector.memset(vpe1[:], 0.0)
        vpe0_3d = vpe0[:].rearrange("p (t c) -> p t c", c=128)
        vpe1_3d = vpe1[:].rearrange("p (t c) -> p t c", c=128)
        nc.vector.memset(vpe0_3d[:, :, C : C + 1], 1.0)
        nc.vector.memset(vpe1_3d[:, :, C : C + 1], 1.0)
        nc.vector.memset(ones_sb[:].bitcast(f32), 1.0)

        nc.sync.dma_start(
            wq_sb[:].rearrange("p (t c) -> p t c", c=CW),
            wq[:, :].rearrange("(t p) c -> p t c", p=128),
        )
        nc.sync.dma_start(
            wk_sb[:].rearrange("p (t c) -> p t c", c=CW),
            wk[:, :].rearrange("(t p) c -> p t c", p=128),
        )
        nc.sync.dma_start(
            wv_sb[:].rearrange("p (t c) -> p t c", c=CW),
            wv[:, :].rearrange("(t p) c -> p t c", p=128),
        )
        nc.sync.dma_start(wo_sb[:], wo[:, :])

        # ---- phase 1: projections (streamed over token chunks) ----
        with tc.tile_pool(name="proj_in", bufs=2) as pin, \
             tc.tile_pool(name="proj_ps", bufs=2, space="PSUM") as pps, \
             tc.tile_pool(name="proj_bounce", bufs=2) as pbn:
            for jc in range(NJ):
                cols = slice(jc * NQT, (jc + 1) * NQT)

                qin = pin.tile([128, KQ * NQT], bf16, tag="qin", name="qin")
                nc.sync.dma_start(
                    qin[:].rearrange("p (t n) -> p t n", n=NQT),
                    qT[:, cols].rearrange("(t p) n -> p t n", p=128),
                )
                qp_ps = pps.tile([128, NQT], f32, tag="pp", name="qp_ps")
                for t in range(KQ):
                    nc.tensor.matmul(
                        qp_ps[:],
                        wq_sb[:, t * CW : (t + 1) * CW],
                        qin[:, t * NQT : (t + 1) * NQT],
                        start=(t == 0),
                        stop=(t == KQ - 1),
                    )
                nc.vector.tensor_copy(qpT_sb[:, cols], qp_ps[:])

                kin = pin.tile([128, KC * NQT], bf16, tag="kin", name="kin")
                nc.sync.dma_start(
                    kin[:].rearrange("p (t n) -> p t n", n=NQT),
                    kT[:, cols].rearrange("(t p) n -> p t n", p=128),
                )
                kp_ps = pps.tile([128, NQT], f32, tag="pp", name="kp_ps")
                for t in range(KC):
                    nc.tensor.matmul(
                        kp_ps[:],
                        wk_sb[:, t * CW : (t + 1) * CW],
                        kin[:, t * NQT : (t + 1) * NQT],
                        start=(t == 0),
                        stop=(t == KC - 1),
                    )
                nc.vector.tensor_copy(kpT_sb[:, cols], kp_ps[:])

                vin = pin.tile([128, KC * NQT], bf16, tag="vin", name="vin")
                nc.sync.dma_start(
                    vin[:].rearrange("p (t n) -> p t n", n=NQT),
                    vT[:, cols].rearrange("(t p) n -> p t n", p=128),
                )
                vp_ps = pps.tile([128, NQT], f32, tag="pp", name="vp_ps")
                for t in range(KC):
                    nc.tensor.matmul(
                        vp_ps[:],
                        wv_sb[:, t * CW : (t + 1) * CW],
                        vin[:, t * NQT : (t + 1) * NQT],
                        start=(t == 0),
                        stop=(t == KC - 1),
                    )
                vpc = pbn.tile([128, NQT], bf16, tag="vpc", name="vpc")
                nc.vector.tensor_copy(vpc[:], vp_ps[:])
                # transpose each [128ch, 128tok] block -> token-major
                for i in range(NQT // 128):
                    t_ps = pps.tile([128, 128], bf16, tag="tp", name="t_ps")
                    nc.tensor.transpose(t_ps[:], vpc[:, i * 128 : (i + 1) * 128], ident[:])
                    kt = jc * (NQT // 128) + i  # key tile index 0..15
                    nc.vector.tensor_copy(
                        vpe0[:, kt * 128 : kt * 128 + C], t_ps[:, 0:C]
                    )
                    nc.vector.tensor_copy(
                        vpe1[:, kt * 128 : kt * 128 + C], t_ps[:, C : 2 * C]
                    )

        # ---- phase 2: attention + fused output projection ----
        vpe = (vpe0, vpe1)
        with tc.tile_pool(name="att_s", bufs=2, space="PSUM") as sps, \
             tc.tile_pool(name="att_x", bufs=1, space="PSUM") as xps, \
             tc.tile_pool(name="att_o", bufs=2, space="PSUM") as ops, \
             tc.tile_pool(name="att_e", bufs=4) as eps, \
             tc.tile_pool(name="att_r", bufs=2) as rps, \
             tc.tile_pool(name="out_bn", bufs=2) as obn:
            for j in range(NJ):
                cols = slice(j * NQT, (j + 1) * NQT)
                x_ps = [
                    xps.tile([128, NQT], f32, tag=f"x{h}", name=f"x_ps{h}")
                    for h in range(HPC)
                ]
                for t in range(NT):
                    # both heads' S^T tiles into one 2-bank PSUM tile;
                    # the two K=64 matmuls row-pack and run concurrently
                    s_ps = sps.tile([128, 2 * NQT], f32, tag="s", name="s_ps")
                    nc.tensor.matmul(
                        s_ps[:, 0:NQT],
                        kpT_sb[0:C, t * NKT : (t + 1) * NKT],
                        qpT_sb[0:C, cols],
                        start=True,
                        stop=True,
                    )
                    nc.tensor.matmul(
                        s_ps[:, NQT : 2 * NQT],
                        kpT_sb[C : 2 * C, t * NKT : (t + 1) * NKT],
                        qpT_sb[C : 2 * C, cols],
                        start=True,
                        stop=True,
                    )
                    # one exp instruction covers both heads (both banks)
                    e_sb = eps.tile([128, 2 * NQT], bf16, tag="e", name="e_sb")
                    nc.scalar.activation(e_sb[:], s_ps[:], AF.Exp)
                    for h in range(HPC):
                        nc.tensor.matmul(
                            x_ps[h][:],
                            vpe[h][:, t * 128 : (t + 1) * 128],
                            e_sb[:, h * NQT : (h + 1) * NQT],
                            start=(t == 0),
                            stop=(t == NT - 1),
                            skip_group_check=True,
                        )
                # normalize: broadcast each head's denominator row across C
                # partitions via a K=1 matmul, then a DVE divide
                for h in range(HPC):
                    sums_sb = rps.tile([1, NQT], f32r, tag="r", name="sums_sb")
                    nc.vector.tensor_copy(sums_sb[:], x_ps[h][C : C + 1, :])
                    b_ps = ops.tile([C, NQT], f32, tag="o", name="b_ps")
                    nc.tensor.matmul(
                        b_ps[:],
                        ones_sb[0:1, :],
                        sums_sb[0:1, :],
                        start=True,
                        stop=True,
                    )
                    b_sb = rps.tile([C, NQT], f32, tag="bsb", name="b_sb")
                    nc.vector.reciprocal_approx_fast(out=b_sb[:], in_=b_ps[:])
                    nc.vector.tensor_mul(
                        xT_sb[h * C : (h + 1) * C, cols],
                        x_ps[h][0:C, :],
                        b_sb[:],
                    )
                # output projection for this query chunk (partial over heads)
                o_sb = obn.tile([128, (DQ // 128) * NQT], bf16, tag="ob", name="o_sb")
                for m in range(DQ // 128):
                    o_ps = ops.tile([128, NQT], f32, tag="o", name="o_ps")
                    nc.tensor.matmul(
                        o_ps[:],
                        wo_sb[:, m * 128 : (m + 1) * 128],
                        xT_sb[:, cols],
                        start=True,
                        stop=True,
                    )
                    nc.vector.tensor_copy(
                        o_sb[:, m * NQT : (m + 1) * NQT], o_ps[:]
                    )
                nc.sync.dma_start(
                    outT[:, cols].rearrange("(m p) n -> p m n", p=128),
                    o_sb[:].rearrange("p (m n) -> p m n", n=NQT),
                )

    nc.compile()
    return nc


def _get_nc():
    if "nc" not in _CACHE:
        _CACHE["nc"] = _build()
    return _CACHE["nc"]


def _round_f32r(x):
    """Round fp32 to the fp32r grid (sign + 8e + 11m: top 20 bits, RNE).

    The PE's fp32r datapath requires pre-rounded inputs; engine writes with
    an fp32r output dtype round in hardware, but DMA-fed tensors must be
    rounded on the host.
    """
    b = np.ascontiguousarray(x, np.float32).view(np.uint32)
    lsb = (b >> np.uint32(12)) & np.uint32(1)
    rounded = (b + np.uint32(0x7FF) + lsb) & np.uint32(0xFFFFF000)
    return rounded.view(np.float32)


def _prep_in_maps(q, k, v, Wq, Wk, Wv, Wo):
    import ml_dtypes

    bf16 = ml_dtypes.bfloat16
    scale = np.float32(C**-0.5)
    qT = np.ascontiguousarray(np.asarray(q, np.float32).T).astype(bf16)
    kT = np.ascontiguousarray(np.asarray(k, np.float32).T).astype(bf16)
    vT = np.ascontiguousarray(np.asarray(v, np.float32).T).astype(bf16)
    Wq = np.asarray(Wq, np.float32)
    Wk = np.asarray(Wk, np.float32)
    Wv = np.asarray(Wv, np.float32)
    Wo = np.asarray(Wo, np.float32)

    in_maps = []
    for i in range(NCORES):
        sl = slice(i * CW, (i + 1) * CW)
        in_maps.append(
            {
                "qT": qT,
                "kT": kT,
                "vT": vT,
                "wq": np.ascontiguousarray(Wq[:, sl] * scale).astype(bf16),
                "wk": np.ascontiguousarray(Wk[:, sl]).astype(bf16),
                "wv": np.ascontiguousarray(Wv[:, sl]).astype(bf16),
                "wo": np.ascontiguousarray(Wo[sl, :]).astype(bf16),
            }
        )
    return in_maps


def kernel(q, k, v, Wq, Wk, Wv, Wo, bo):
    from concourse.bass_utils import run_bass_kernel_spmd

    bo = np.asarray(bo, np.float32)
    in_maps = _prep_in_maps(q, k, v, Wq, Wk, Wv, Wo)
    nc = _get_nc()
    res = run_bass_kernel_spmd(nc, in_maps, list(range(NCORES)))
    acc = res.results[0]["outT"].astype(np.float32)
    for i in range(1, NCORES):
        acc = acc + res.results[i]["outT"].astype(np.float32)
    return (acc.T + bo[None, :]).astype(np.float32)


if __name__ == "__main__":
    rng = np.random.default_rng(0)
    q = rng.standard_normal((NQ, DQ), np.float32)
    k = rng.standard_normal((NK, DC), np.float32)
    v = rng.standard_normal((NK, DC), np.float32)
    Wq = rng.standard_normal((DQ, DQ), np.float32) * 0.02
    Wk = rng.standard_normal((DC, DQ), np.float32) * 0.02
    Wv = rng.standard_normal((DC, DQ), np.float32) * 0.02
    Wo = rng.standard_normal((DQ, DQ), np.float32) * 0.02
    bo = np.zeros((DQ,), np.float32)
    out = kernel(q=q, k=k, v=v, Wq=Wq, Wk=Wk, Wv=Wv, Wo=Wo, bo=bo)
    print(out.shape, out.dtype, np.abs(out).mean())
